# revision 15
# baseline (speedup 1.0000x reference)
"""Cross-attention kernel for Trainium2, SPMD over 8 NeuronCores.

Problem: T=4, B=2, NQ=NK=1024, C=512, H=8 heads (D=64).
  q = clip01(BN0(query @ Wq.T)); k = clip01(BN1(key @ Wk.T)); v = clip01(BN2(value @ Wv.T))
  per head: O = softmax(Q K^T / sqrt(D)) V
  out = BN3(concat(O) @ Wo.T)

Sharding: pure data-parallel, one (t, b) pair per core (T*B == 8 == n_cores).

Design notes (cost model: matmul = out-free-rows * cycles_per_row; fp8
DoubleRow = 0.5 cycles/row with two contraction slots per instruction):
  - host quantizes x and BN-folded weights to fp8e4m3 (error budget ~0.4%
    measured end-to-end; threshold 2e-2). Inputs stream as fp8 -> tiny DMA.
  - q/k/v projections: fp8 DoubleRow over c-tile pairs (K=128x2).
  - q is scaled by 1/8 (folded into Wq'/bias; clip to [0, 0.125]) so the
    score matmul directly yields S' = S/8.
  - scores: fp8 DoubleRow with slot0 = real d-contraction (K=64), slot1 =
    constants (q-side 0.04296875, k-side -1.0) contributing -2.75, so PSUM
    holds S/8 - 2.75 and exp needs no scale/bias anywhere. The shift
    cancels in softmax; it keeps exp output <= e^5.25 = 191 < fp8 max 240.
  - exp: batched [128, 1024] over 2-bank PSUM pair tiles; split between
    the ACT engine (native Exp) and Pool/GPSIMD (tensor_tensor pow with a
    broadcast e base) to beat the single-engine exp floor. Output fp8.
  - PV: fp8 DoubleRow, m-tile pairs as the two slots; V carries a ones
    column per (m, h) so U row 64 accumulates the softmax denominator.
  - normalize: recip + partition-broadcast + DVE mul into oT (f32r).
  - out-projection: f32r, accumulated across the 4 d-tiles in PSUM
    (ones-row matmul adds the bias), one DVE copy, DMA out per m-tile.
"""

import numpy as np

import os

H, D, C, N = 8, 64, 512, 1024
CT = C // 128          # 4 c-tiles
NT = N // 128          # 8 n-tiles
CH = N // 512          # 2 free-dim chunks of 512
EPS = 1e-5
N_CORES = 8

# exp(S/8 - CSHIFT); CSHIFT = 64 * CQ * CK must keep e^(8-CSHIFT) < 240
CQ = 0.04296875        # q-side const slot value (exact in fp8e4m3)
CK = -1.0              # k-side const slot value
CSHIFT = 2.75

POOL_EXP = int(os.environ.get("K_PO", "0"))   # of 64 exp pairs, run on Pool
MINQ_DVE = int(os.environ.get("K_MINQ", "8"))  # of 16 q/k clip-mins, on DVE
EP_BUFS = int(os.environ.get("K_EP", "6"))
PSQ_BUFS = int(os.environ.get("K_PSQ", "2"))
WARM = int(os.environ.get("K_WARM", "6"))

_CACHE = {}


def _build():
    from contextlib import ExitStack

    import concourse.bass as bass
    import concourse.tile as tile
    from concourse import bacc, mybir

    f32 = mybir.dt.float32
    f32r = mybir.dt.float32r
    fp8 = mybir.dt.float8e4
    ts = bass.ts
    Exp = mybir.ActivationFunctionType.Exp
    DR = mybir.MatmulPerfMode.DoubleRow
    MAX, MIN = mybir.AluOpType.max, mybir.AluOpType.min
    POW = mybir.AluOpType.pow

    nc = bacc.Bacc("TRN2", target_bir_lowering=False, debug=False,
                   num_devices=N_CORES)

    xq = nc.dram_tensor("xq", [C, N], fp8, kind="ExternalInput").ap()
    xk = nc.dram_tensor("xk", [C, N], fp8, kind="ExternalInput").ap()
    xv = nc.dram_tensor("xv", [C, N], fp8, kind="ExternalInput").ap()
    wq = nc.dram_tensor("wq", [C, C], fp8, kind="ExternalInput").ap()
    wk = nc.dram_tensor("wk", [C, C], fp8, kind="ExternalInput").ap()
    wv = nc.dram_tensor("wv", [C, C], fp8, kind="ExternalInput").ap()
    wo = nc.dram_tensor("wo", [C, C], f32r, kind="ExternalInput").ap()
    tbias = nc.dram_tensor("tbias", [4, C], f32r, kind="ExternalInput").ap()
    tbt = nc.dram_tensor("tbt", [128, 8], f32, kind="ExternalInput").ap()
    out = nc.dram_tensor("out", [N, C], f32, kind="ExternalOutput").ap()

    def mmDR(out_ap, lhsT, rhs, **kw):
        nc.tensor.matmul(out_ap, lhsT=lhsT, rhs=rhs, perf_mode=DR,
                         skip_group_check=True, **kw)

    def mm(out_ap, lhsT, rhs, **kw):
        nc.tensor.matmul(out_ap, lhsT=lhsT, rhs=rhs,
                         skip_group_check=True, **kw)

    with tile.TileContext(nc) as tc, ExitStack() as ctx:
        sb = ctx.enter_context(tc.tile_pool(name="sb", bufs=1))
        ep = ctx.enter_context(tc.tile_pool(name="ep", bufs=EP_BUFS))
        yp = ctx.enter_context(tc.tile_pool(name="yp", bufs=3))
        psq = ctx.enter_context(tc.tile_pool(name="psq", bufs=PSQ_BUFS,
                                             space="PSUM"))
        pj = ctx.enter_context(tc.tile_pool(name="pj", bufs=2, space="PSUM"))
        up = ctx.enter_context(tc.tile_pool(name="up", bufs=2, space="PSUM"))

        ones = sb.tile([1, N], f32r, tag="ones")
        nc.gpsimd.memset(ones[:].bitcast(f32), 1.0)
        ebase = sb.tile([128, 1], f32, tag="ebase")
        nc.gpsimd.memset(ebase[:], float(np.e))
        # warm the ACT exp table while input DMAs stream
        junk = sb.tile([1, 8], f32, tag="junk")
        nc.scalar.activation(junk[:], ones[0:1, 0:8].bitcast(f32), Exp)
        for w in range(WARM):
            pw = pj.tile([128, 512], f32, tag="pj", name=f"warm{w}")
            mm(pw[:], ones[0:1, 0:128], rhs=ones[0:1, 0:512].bitcast(f32r),
               start=True, stop=True)

        # q/k epilogue biases first (tiny, gate the first projections);
        # the V/O bias rows ride the Pool software DGE off the critical path
        tbt_sb = sb.tile([128, 8], f32, tag="tbt")
        nc.sync.dma_start(tbt_sb[:], tbt[:])
        tbs = sb.tile([1, 4 * C], f32r, tag="tbs")
        nc.gpsimd.dma_start(tbs[:].rearrange("p (j c) -> p j c", c=C),
                            tbias[:].rearrange("(j p) c -> p j c", p=1))
        tb = [tbs[0:1, ts(j, C)] for j in range(4)]

        def load_w(name, ap, dt):
            t = sb.tile([128, CT * C], dt, tag=name, name=name)
            nc.sync.dma_start(t[:].rearrange("p (c d) -> p c d", d=C),
                              ap[:].rearrange("(c p) d -> p c d", p=128))
            return t

        def load_x_alloc(name, ap):
            t = sb.tile([128, CT * N], fp8, tag=name, name=name)
            tv = t[:].rearrange("p (c n) -> p c n", n=N)
            av = ap[:].rearrange("(c p) n -> p c n", p=128)
            def half(ch):
                nc.sync.dma_start(tv[:, :, ts(ch, 512)], av[:, :, ts(ch, 512)])
            return t, half

        # order: everything the first (hp0, ch0) scores need comes first
        wq_t = load_w("wq", wq, fp8)
        xq_t, xq_half = load_x_alloc("xq", xq)
        xq_half(0)
        wk_t = load_w("wk", wk, fp8)
        xk_t, xk_half = load_x_alloc("xk", xk)
        xk_half(0)
        wv_t = load_w("wv", wv, fp8)
        xv_t, xv_half = load_x_alloc("xv", xv)
        xv_half(0)
        xq_half(1)
        xk_half(1)
        xv_half(1)
        wo_t = load_w("wo", wo, f32r)
        wo_v = wo_t[:].rearrange("p (c d) -> p c d", d=C)

        # q/k fp8 tiles with const second slot: [128, 2 slots, N]
        qk_tiles = {}
        for nm, cv in (("qA", CQ), ("qB", CQ), ("kA", CK), ("kB", CK)):
            t = sb.tile([128, 2 * N], fp8, tag=nm, name=nm)
            nc.gpsimd.memset(t[:, N:2 * N], cv)
            qk_tiles[nm] = t

        # V with ones column: free layout [mp 4][slot 2][h 8][66]
        # (66 pads the per-head block so the DoubleRow slot stride 8*66=528
        #  is 16-byte aligned, an ISA requirement for dual-fp8 ldweights)
        DP = D + 2
        V_all = sb.tile([128, NT * H * DP], fp8, tag="V_all")
        V_v = V_all[:].rearrange("p (m s h d) -> p m s h d", m=4, s=2, d=DP)
        nc.gpsimd.memset(V_v[:, :, :, :, D:D + 1], 1.0)

        # oT: normalized attention output, [C, N] transposed (f32r)
        oT = [sb.tile([128, N], f32r, tag=f"oT{j}", name=f"oT{j}")
              for j in range(CT)]

        wv_v = wv_t[:].rearrange("p (c d) -> p c d", d=C)
        xv_v = xv_t[:].rearrange("p (c n) -> p c n", n=N)

        state = {"exp_i": 0, "minq_i": 0}

        def proj_t(w_t, x_t, bias_col, j, dst, minv, chunks=(0, 1),
                   split_epi=False):
            """Transposed projection d-tile j -> fp8 slot 0 of dst."""
            w_v = w_t[:].rearrange("p (ct two d) -> p ct two d", ct=2, two=2)
            x_v = x_t[:].rearrange("p (ct two n) -> p ct two n", ct=2, two=2)
            for ch in chunks:
                p = pj.tile([128, 512], f32, tag="pj")
                for cp in range(2):
                    mmDR(p[:], w_v[:, cp, :, ts(j, 128)],
                         rhs=x_v[:, cp, :, ts(ch, 512)],
                         start=(cp == 0), stop=(cp == 1))
                spans = ((0, 256), (256, 512)) if split_epi else ((0, 512),)
                for lo, hi in spans:
                    d = dst[:, ch * 512 + lo:ch * 512 + hi]
                    nc.vector.tensor_scalar(d, p[:, lo:hi], bias_col, 0.0,
                                            mybir.AluOpType.add, MAX)
                    if state["minq_i"] % 16 < MINQ_DVE:
                        nc.vector.tensor_scalar_min(d, d, minv)
                    else:
                        nc.gpsimd.tensor_scalar_min(d, d, minv)
                state["minq_i"] += 1

        def v_proj(m):
            p = pj.tile([128, 512], f32, tag="pj")
            for cp in range(2):
                mmDR(p[:], xv_v[:, :, ts(m, 128)].rearrange(
                         "p (ct two) n -> p ct two n", ct=2)[:, cp],
                     rhs=wv_v[:, :, :].rearrange(
                         "p (ct two) d -> p ct two d", ct=2)[:, cp],
                     start=(cp == 0), stop=False)
            mm(p[:], ones[0:1, 0:128], rhs=tb[2], start=False, stop=True)
            nc.vector.tensor_scalar(V_v[:, m // 2, m % 2, :, 0:D],
                                    p[:].rearrange("p (h d) -> p h d", d=D),
                                    0.0, 1.0, MAX, MIN)

        def qk_slots(t):
            return t[:].rearrange("p (s n) -> p s n", s=2)

        def exp_pair(PQ, E8):
            # PQ: [128, 2w] f32 PSUM slice; E8: [128, 2w] fp8 SBUF slice
            i = state["exp_i"]
            state["exp_i"] += 1
            n = PQ.shape[-1]
            if i % 8 < POOL_EXP // 8:
                # Pool cannot read PSUM: stage via DVE copy, then pow on Pool
                st = ep.tile([128, 1024], f32, tag="st", name=f"st{i}")
                nc.vector.tensor_copy(st[:, 0:n], PQ)
                nc.gpsimd.tensor_tensor(E8, ebase[:].broadcast_to((128, n)),
                                        st[:, 0:n], POW)
            else:
                nc.scalar.activation(E8, PQ, Exp)

        def out_proj(m):
            p = pj.tile([128, 512], f32, tag="pj")
            for ck in range(CT):
                mm(p[:], oT[ck][:, ts(m, 128)], rhs=wo_v[:, ck, :],
                   start=(ck == 0), stop=False)
            mm(p[:], ones[0:1, 0:128], rhs=tb[3], start=False, stop=True)
            y = yp.tile([128, 512], f32, tag="y", bufs=4)
            nc.vector.tensor_copy(y[:], p[:])
            nc.sync.dma_start(out[ts(m, 128), :], y[:])

        # tail m-tiles (4..7): j=0..2 partials accumulated early in SBUF so
        # only (j3 + bias + add + DMA) remains after the last normalize
        y_acc = sb.tile([128, 4 * 512], f32, tag="y_acc")

        def out_proj_partial(m):
            p = pj.tile([128, 512], f32, tag="pj")
            for ck in range(3):
                mm(p[:], oT[ck][:, ts(m, 128)], rhs=wo_v[:, ck, :],
                   start=(ck == 0), stop=(ck == 2))
            nc.vector.tensor_copy(y_acc[:, ts(m - 4, 512)], p[:])

        def out_proj_final(m):
            p = pj.tile([128, 512], f32, tag="pj")
            mm(p[:], oT[3][:, ts(m, 128)], rhs=wo_v[:, 3, :],
               start=True, stop=False)
            mm(p[:], ones[0:1, 0:128], rhs=tb[3], start=False, stop=True)
            y = yp.tile([128, 512], f32, tag="y", bufs=4)
            nc.vector.tensor_tensor(y[:], p[:], y_acc[:, ts(m - 4, 512)],
                                    mybir.AluOpType.add)
            nc.sync.dma_start(out[ts(m, 128), :], y[:])

        def attention(hp, qT, kT):
            heads = (2 * hp, 2 * hp + 1)
            qs, ks = qk_slots(qT), qk_slots(kT)
            # hp3's second half runs as two quarter-chunks so the post-exp
            # tail (normalize + out-proj + DMA) only covers 2 m-tiles
            subs = ([(0, 512), (512, 512)] if hp < 3 else
                    [(0, 512), (512, 256), (768, 256)])
            for ci, (off, w) in enumerate(subs):
                U = {h: up.tile([D + 1, 512], f32, tag="U",
                                name=f"U{h}_{ci}") for h in heads}
                for mp in range(4):
                    E8s = {}
                    for h in heads:
                        base = (h % 2) * 64
                        PQ = psq.tile([128, 1024], f32, tag="psq")
                        for s in range(2):
                            m = 2 * mp + s
                            mmDR(PQ[:, s * w:(s + 1) * w],
                                 ks[base:base + 64, :, ts(m, 128)],
                                 rhs=qs[base:base + 64, :, off:off + w],
                                 start=True, stop=True)
                        E8 = ep.tile([128, 1024], fp8, tag="E8",
                                     name=f"E8_{h}_{mp}_{ci}")
                        exp_pair(PQ[:, 0:2 * w], E8[:, 0:2 * w])
                        E8s[h] = E8
                    if hp == 0 and ci == 0:
                        v_proj(2 * mp)
                        v_proj(2 * mp + 1)
                    for h in heads:
                        mmDR(U[h][:, 0:w], V_v[:, mp, :, h, 0:D + 1],
                             rhs=E8s[h][:, 0:2 * w].rearrange(
                                 "p (s q) -> p s q", s=2),
                             start=(mp == 0), stop=(mp == 3))
                    # interleave remaining chunk-1 projections of q0/k0 and
                    # next-phase projections / out-proj partials
                    if hp == 0 and ci == 0 and mp == 1:
                        proj_t(wq_t, xq_t, tbt_sb[:, 0:1], 0, qT, 0.125,
                               chunks=(1,))
                    if hp == 0 and ci == 0 and mp == 3:
                        proj_t(wk_t, xk_t, tbt_sb[:, 4:5], 0, kT, 1.0,
                               chunks=(1,))
                    if hp < 3 and mp == 2:
                        if ci == 0:
                            proj_t(wq_t, xq_t, tbt_sb[:, hp + 1:hp + 2],
                                   hp + 1, nxt["q"], 0.125)
                        else:
                            proj_t(wk_t, xk_t, tbt_sb[:, 5 + hp:6 + hp],
                                   hp + 1, nxt["k"], 1.0)
                    if hp == 3 and ci == 0:
                        out_proj_partial(mp + 4)
                    if hp == 3 and ci == 1:
                        out_proj(mp)
                # per-chunk softmax normalization (denominator in U row D)
                for h in heads:
                    rc = yp.tile([1, 512], f32, tag="rc")
                    nc.vector.reciprocal(rc[:, 0:w], U[h][D:D + 1, 0:w])
                    B = yp.tile([64, 512], f32, tag="B")
                    nc.gpsimd.partition_broadcast(B[:, 0:w], rc[0:1, 0:w],
                                                  channels=64)
                    base = (h % 2) * 64
                    nc.vector.tensor_mul(oT[hp][base:base + 64, off:off + w],
                                         U[h][0:D, 0:w], B[:, 0:w])
                if hp == 3 and ci >= 1:
                    out_proj_final(off // 128)
                    out_proj_final(off // 128 + 1)

        nxt = {"q": qk_tiles["qA"], "k": qk_tiles["kA"]}
        proj_t(wq_t, xq_t, tbt_sb[:, 0:1], 0, nxt["q"], 0.125, chunks=(0,))
        proj_t(wk_t, xk_t, tbt_sb[:, 4:5], 0, nxt["k"], 1.0, chunks=(0,),
               split_epi=True)
        for hp in range(4):
            cur_q, cur_k = nxt["q"], nxt["k"]
            nxt = {"q": qk_tiles["qB" if hp % 2 == 0 else "qA"],
                   "k": qk_tiles["kB" if hp % 2 == 0 else "kA"]}
            attention(hp, cur_q, cur_k)

    nc.compile()
    return nc


def get_nc():
    if "nc" not in _CACHE:
        _CACHE["nc"] = _build()
    return _CACHE["nc"]


def _prep_inputs(query, key, value, Wq, Wk, Wv, Wo, bn_params):
    """Host-side: shard + transpose + fold BN scale into weights + fp8."""
    import ml_dtypes

    f8 = ml_dtypes.float8_e4m3

    query = np.ascontiguousarray(np.asarray(query, dtype=np.float32))
    key = np.ascontiguousarray(np.asarray(key, dtype=np.float32))
    value = np.ascontiguousarray(np.asarray(value, dtype=np.float32))
    bn = np.asarray(bn_params, dtype=np.float32)

    s = bn[:, 0] / np.sqrt(bn[:, 3] + EPS)      # [4, C]
    t = bn[:, 1] - bn[:, 2] * s                  # [4, C]

    def wprep(W, j, scale=1.0):
        W = np.asarray(W, dtype=np.float32)
        return np.ascontiguousarray((W * (s[j] * scale)[:, None]).T)

    wq8 = wprep(Wq, 0, 0.125).astype(f8)
    wk8 = wprep(Wk, 1).astype(f8)
    wv8 = wprep(Wv, 2).astype(f8)
    woT = wprep(Wo, 3)
    tbias = np.ascontiguousarray(t)
    # transposed q/k biases: rows (proj, d-tile) of 128; q scaled by 1/8
    tbt = np.ascontiguousarray(
        np.concatenate([(t[0] * 0.125).reshape(4, 128),
                        t[1].reshape(4, 128)]).T)

    # [T, B, N, C] -> [8, C, N] fp8
    def xT(x):
        return np.ascontiguousarray(
            x.reshape(N_CORES, N, C).transpose(0, 2, 1)).astype(f8)

    qT, kT, vT = xT(query), xT(key), xT(value)

    in_maps = []
    for i in range(N_CORES):
        in_maps.append({
            "xq": qT[i], "xk": kT[i], "xv": vT[i],
            "wq": wq8, "wk": wk8, "wv": wv8, "wo": woT,
            "tbias": tbias, "tbt": tbt,
        })
    return in_maps


def kernel(query, key, value, Wq, Wk, Wv, Wo, bn_params):
    from concourse.bass_utils import run_bass_kernel_spmd

    nc = get_nc()
    in_maps = _prep_inputs(query, key, value, Wq, Wk, Wv, Wo, bn_params)
    res = run_bass_kernel_spmd(nc, in_maps, core_ids=list(range(N_CORES)),
                               trace=False)
    T, B = 4, 2
    out = np.stack([res.results[i]["out"] for i in range(N_CORES)])
    return np.ascontiguousarray(out.reshape(T, B, N, C).astype(np.float32))


# revision 17
# speedup vs baseline: 1.3314x; 1.3314x over previous
"""Cross-attention kernel for Trainium2, SPMD over 8 NeuronCores.

Problem: T=4, B=2, NQ=NK=1024, C=512, H=8 heads (D=64).
  q = clip01(BN0(query @ Wq.T)); k = clip01(BN1(key @ Wk.T)); v = clip01(BN2(value @ Wv.T))
  per head: O = softmax(Q K^T / sqrt(D)) V
  out = BN3(concat(O) @ Wo.T)

Sharding: pure data-parallel, one (t, b) pair per core (T*B == 8 == n_cores).

Design notes (cost model: matmul = out-free-rows * cycles_per_row; fp8
DoubleRow = 0.5 cycles/row with two contraction slots per instruction):
  - host quantizes x and BN-folded weights to fp8e4m3 (error budget ~0.4%
    measured end-to-end; threshold 2e-2). Inputs stream as fp8 -> tiny DMA.
  - q/k/v projections: fp8 DoubleRow over c-tile pairs (K=128x2).
  - q is scaled by 1/8 (folded into Wq'/bias; clip to [0, 0.125]) so the
    score matmul directly yields S' = S/8.
  - scores: fp8 DoubleRow with slot0 = real d-contraction (K=64), slot1 =
    constants (q-side 0.04296875, k-side -1.0) contributing -2.75, so PSUM
    holds S/8 - 2.75 and exp needs no scale/bias anywhere. The shift
    cancels in softmax; it keeps exp output <= e^5.25 = 191 < fp8 max 240.
  - exp: batched [128, 1024] over 2-bank PSUM pair tiles; split between
    the ACT engine (native Exp) and Pool/GPSIMD (tensor_tensor pow with a
    broadcast e base) to beat the single-engine exp floor. Output fp8.
  - PV: fp8 DoubleRow, m-tile pairs as the two slots; V carries a ones
    column per (m, h) so U row 64 accumulates the softmax denominator.
  - normalize: recip + partition-broadcast + DVE mul into oT (f32r).
  - out-projection: f32r, accumulated across the 4 d-tiles in PSUM
    (ones-row matmul adds the bias), one DVE copy, DMA out per m-tile.
"""

import numpy as np

import os

H, D, C, N = 8, 64, 512, 1024
CT = C // 128          # 4 c-tiles
NT = N // 128          # 8 n-tiles
CH = N // 512          # 2 free-dim chunks of 512
EPS = 1e-5
N_CORES = 8

# exp(S/8 - CSHIFT); CSHIFT = 64 * CQ * CK must keep e^(8-CSHIFT) < 240
CQ = 0.04296875        # q-side const slot value (exact in fp8e4m3)
CK = -1.0              # k-side const slot value
CSHIFT = 2.75

POOL_EXP = int(os.environ.get("K_PO", "0"))   # of 64 exp pairs, run on Pool
MINQ_DVE = int(os.environ.get("K_MINQ", "8"))  # of 16 q/k clip-mins, on DVE
EP_BUFS = int(os.environ.get("K_EP", "6"))
PSQ_BUFS = int(os.environ.get("K_PSQ", "2"))
WARM = int(os.environ.get("K_WARM", "6"))

_CACHE = {}


def _build():
    from contextlib import ExitStack

    import concourse.bass as bass
    import concourse.tile as tile
    from concourse import bacc, mybir

    f32 = mybir.dt.float32
    f32r = mybir.dt.float32r
    fp8 = mybir.dt.float8e4
    ts = bass.ts
    Exp = mybir.ActivationFunctionType.Exp
    DR = mybir.MatmulPerfMode.DoubleRow
    MAX, MIN = mybir.AluOpType.max, mybir.AluOpType.min
    POW = mybir.AluOpType.pow

    nc = bacc.Bacc("TRN2", target_bir_lowering=False, debug=False,
                   num_devices=N_CORES)

    xq = nc.dram_tensor("xq", [C, N], fp8, kind="ExternalInput").ap()
    xk = nc.dram_tensor("xk", [C, N], fp8, kind="ExternalInput").ap()
    xv = nc.dram_tensor("xv", [C, N], fp8, kind="ExternalInput").ap()
    wq = nc.dram_tensor("wq", [C, C], fp8, kind="ExternalInput").ap()
    wk = nc.dram_tensor("wk", [C, C], fp8, kind="ExternalInput").ap()
    wv = nc.dram_tensor("wv", [C, C], fp8, kind="ExternalInput").ap()
    wo = nc.dram_tensor("wo", [C, C], f32r, kind="ExternalInput").ap()
    tbias = nc.dram_tensor("tbias", [4, C], f32r, kind="ExternalInput").ap()
    tbt = nc.dram_tensor("tbt", [128, 8], f32, kind="ExternalInput").ap()
    out = nc.dram_tensor("out", [N, C], f32, kind="ExternalOutput").ap()

    def mmDR(out_ap, lhsT, rhs, **kw):
        nc.tensor.matmul(out_ap, lhsT=lhsT, rhs=rhs, perf_mode=DR,
                         skip_group_check=True, **kw)

    def mm(out_ap, lhsT, rhs, **kw):
        nc.tensor.matmul(out_ap, lhsT=lhsT, rhs=rhs,
                         skip_group_check=True, **kw)

    with tile.TileContext(nc) as tc, ExitStack() as ctx:
        sb = ctx.enter_context(tc.tile_pool(name="sb", bufs=1))
        ep = ctx.enter_context(tc.tile_pool(name="ep", bufs=EP_BUFS))
        yp = ctx.enter_context(tc.tile_pool(name="yp", bufs=3))
        psq = ctx.enter_context(tc.tile_pool(name="psq", bufs=PSQ_BUFS,
                                             space="PSUM"))
        pj = ctx.enter_context(tc.tile_pool(name="pj", bufs=2, space="PSUM"))
        up = ctx.enter_context(tc.tile_pool(name="up", bufs=2, space="PSUM"))

        ones = sb.tile([1, N], f32r, tag="ones")
        nc.gpsimd.memset(ones[:].bitcast(f32), 1.0)
        ebase = sb.tile([128, 1], f32, tag="ebase")
        nc.gpsimd.memset(ebase[:], float(np.e))
        # warm the ACT exp table while input DMAs stream
        junk = sb.tile([1, 8], f32, tag="junk")
        nc.scalar.activation(junk[:], ones[0:1, 0:8].bitcast(f32), Exp)
        for w in range(WARM):
            pw = pj.tile([128, 512], f32, tag="pj", name=f"warm{w}")
            mm(pw[:], ones[0:1, 0:128], rhs=ones[0:1, 0:512].bitcast(f32r),
               start=True, stop=True)

        # q/k epilogue biases first (tiny, gate the first projections);
        # the V/O bias rows ride the Pool software DGE off the critical path
        tbt_sb = sb.tile([128, 8], f32, tag="tbt")
        nc.sync.dma_start(tbt_sb[:], tbt[:])
        tbs = sb.tile([1, 4 * C], f32r, tag="tbs")
        nc.gpsimd.dma_start(tbs[:].rearrange("p (j c) -> p j c", c=C),
                            tbias[:].rearrange("(j p) c -> p j c", p=1))
        tb = [tbs[0:1, ts(j, C)] for j in range(4)]

        def load_w(name, ap, dt):
            t = sb.tile([128, CT * C], dt, tag=name, name=name)
            nc.sync.dma_start(t[:].rearrange("p (c d) -> p c d", d=C),
                              ap[:].rearrange("(c p) d -> p c d", p=128))
            return t

        def load_x_alloc(name, ap):
            t = sb.tile([128, CT * N], fp8, tag=name, name=name)
            tv = t[:].rearrange("p (c n) -> p c n", n=N)
            av = ap[:].rearrange("(c p) n -> p c n", p=128)
            def half(ch):
                nc.sync.dma_start(tv[:, :, ts(ch, 512)], av[:, :, ts(ch, 512)])
            return t, half

        # order: everything the first (hp0, ch0) scores need comes first
        wq_t = load_w("wq", wq, fp8)
        xq_t, xq_half = load_x_alloc("xq", xq)
        xq_half(0)
        wk_t = load_w("wk", wk, fp8)
        xk_t, xk_half = load_x_alloc("xk", xk)
        xk_half(0)
        wv_t = load_w("wv", wv, fp8)
        xv_t, xv_half = load_x_alloc("xv", xv)
        xv_half(0)
        xq_half(1)
        xk_half(1)
        xv_half(1)
        wo_t = load_w("wo", wo, f32r)
        wo_v = wo_t[:].rearrange("p (c d) -> p c d", d=C)

        # q/k fp8 tiles with const second slot: [128, 2 slots, N]
        qk_tiles = {}
        for nm, cv in (("qA", CQ), ("qB", CQ), ("kA", CK), ("kB", CK)):
            t = sb.tile([128, 2 * N], fp8, tag=nm, name=nm)
            nc.gpsimd.memset(t[:, N:2 * N], cv)
            qk_tiles[nm] = t

        # V with ones column: free layout [mp 4][slot 2][h 8][66]
        # (66 pads the per-head block so the DoubleRow slot stride 8*66=528
        #  is 16-byte aligned, an ISA requirement for dual-fp8 ldweights)
        DP = D + 2
        V_all = sb.tile([128, NT * H * DP], fp8, tag="V_all")
        V_v = V_all[:].rearrange("p (m s h d) -> p m s h d", m=4, s=2, d=DP)
        nc.gpsimd.memset(V_v[:, :, :, :, D:D + 1], 1.0)

        # oT: normalized attention output, [C, N] transposed (f32r)
        oT = [sb.tile([128, N], f32r, tag=f"oT{j}", name=f"oT{j}")
              for j in range(CT)]

        wv_v = wv_t[:].rearrange("p (c d) -> p c d", d=C)
        xv_v = xv_t[:].rearrange("p (c n) -> p c n", n=N)

        state = {"exp_i": 0, "minq_i": 0}

        def proj_t(w_t, x_t, bias_col, j, dst, minv, chunks=(0, 1),
                   split_epi=False):
            """Transposed projection d-tile j -> fp8 slot 0 of dst."""
            w_v = w_t[:].rearrange("p (ct two d) -> p ct two d", ct=2, two=2)
            x_v = x_t[:].rearrange("p (ct two n) -> p ct two n", ct=2, two=2)
            for ch in chunks:
                p = pj.tile([128, 512], f32, tag="pj")
                for cp in range(2):
                    mmDR(p[:], w_v[:, cp, :, ts(j, 128)],
                         rhs=x_v[:, cp, :, ts(ch, 512)],
                         start=(cp == 0), stop=(cp == 1))
                spans = ((0, 256), (256, 512)) if split_epi else ((0, 512),)
                for lo, hi in spans:
                    d = dst[:, ch * 512 + lo:ch * 512 + hi]
                    nc.vector.tensor_scalar(d, p[:, lo:hi], bias_col, 0.0,
                                            mybir.AluOpType.add, MAX)
                    if state["minq_i"] % 16 < MINQ_DVE:
                        nc.vector.tensor_scalar_min(d, d, minv)
                    else:
                        nc.gpsimd.tensor_scalar_min(d, d, minv)
                state["minq_i"] += 1

        def v_proj(m):
            p = pj.tile([128, 512], f32, tag="pj")
            for cp in range(2):
                mmDR(p[:], xv_v[:, :, ts(m, 128)].rearrange(
                         "p (ct two) n -> p ct two n", ct=2)[:, cp],
                     rhs=wv_v[:, :, :].rearrange(
                         "p (ct two) d -> p ct two d", ct=2)[:, cp],
                     start=(cp == 0), stop=False)
            mm(p[:], ones[0:1, 0:128], rhs=tb[2], start=False, stop=True)
            nc.vector.tensor_scalar(V_v[:, m // 2, m % 2, :, 0:D],
                                    p[:].rearrange("p (h d) -> p h d", d=D),
                                    0.0, 1.0, MAX, MIN)

        def qk_slots(t):
            return t[:].rearrange("p (s n) -> p s n", s=2)

        def exp_pair(PQ, E8):
            # PQ: [128, 2w] f32 PSUM slice; E8: [128, 2w] fp8 SBUF slice
            i = state["exp_i"]
            state["exp_i"] += 1
            n = PQ.shape[-1]
            if i % 8 < POOL_EXP // 8:
                # Pool cannot read PSUM: stage via DVE copy, then pow on Pool
                st = ep.tile([128, 1024], f32, tag="st", name=f"st{i}")
                nc.vector.tensor_copy(st[:, 0:n], PQ)
                nc.gpsimd.tensor_tensor(E8, ebase[:].broadcast_to((128, n)),
                                        st[:, 0:n], POW)
            else:
                nc.scalar.activation(E8, PQ, Exp)

        def out_proj(m):
            p = pj.tile([128, 512], f32, tag="pj")
            for ck in range(CT):
                mm(p[:], oT[ck][:, ts(m, 128)], rhs=wo_v[:, ck, :],
                   start=(ck == 0), stop=False)
            mm(p[:], ones[0:1, 0:128], rhs=tb[3], start=False, stop=True)
            y = yp.tile([128, 512], f32, tag="y", bufs=4)
            nc.vector.tensor_copy(y[:], p[:])
            nc.sync.dma_start(out[ts(m, 128), :], y[:])

        # tail m-tiles (4..7): j=0..2 partials accumulated early in SBUF so
        # only (j3 + bias + add + DMA) remains after the last normalize
        y_acc = sb.tile([128, 4 * 512], f32, tag="y_acc")

        def out_proj_partial(m):
            p = pj.tile([128, 512], f32, tag="pj")
            for ck in range(3):
                mm(p[:], oT[ck][:, ts(m, 128)], rhs=wo_v[:, ck, :],
                   start=(ck == 0), stop=(ck == 2))
            nc.vector.tensor_copy(y_acc[:, ts(m - 4, 512)], p[:])

        def out_proj_final(m):
            p = pj.tile([128, 512], f32, tag="pj")
            mm(p[:], oT[3][:, ts(m, 128)], rhs=wo_v[:, 3, :],
               start=True, stop=False)
            mm(p[:], ones[0:1, 0:128], rhs=tb[3], start=False, stop=True)
            y = yp.tile([128, 512], f32, tag="y", bufs=4)
            nc.vector.tensor_tensor(y[:], p[:], y_acc[:, ts(m - 4, 512)],
                                    mybir.AluOpType.add)
            nc.sync.dma_start(out[ts(m, 128), :], y[:])

        def attention(hp, qT, kT):
            heads = (2 * hp, 2 * hp + 1)
            qs, ks = qk_slots(qT), qk_slots(kT)
            # hp3's second half runs as two quarter-chunks so the post-exp
            # tail (normalize + out-proj + DMA) only covers 2 m-tiles
            if hp < 3:
                subs = [(0, 512), (512, 512)]
            elif int(os.environ.get("K_NOSPLIT", "0")):
                subs = [(0, 512), (512, 512), (512, 512)]  # sentinel unused
                subs = [(0, 512), (512, 512)]
            else:
                subs = [(0, 512), (512, 256), (768, 256)]
            for ci, (off, w) in enumerate(subs):
                U = {h: up.tile([D + 1, 512], f32, tag="U",
                                name=f"U{h}_{ci}") for h in heads}
                for mp in range(4):
                    E8s = {}
                    for h in heads:
                        base = (h % 2) * 64
                        PQ = psq.tile([128, 1024], f32, tag="psq")
                        for s in range(2):
                            m = 2 * mp + s
                            mmDR(PQ[:, s * w:(s + 1) * w],
                                 ks[base:base + 64, :, ts(m, 128)],
                                 rhs=qs[base:base + 64, :, off:off + w],
                                 start=True, stop=True)
                        E8 = ep.tile([128, 1024], fp8, tag="E8",
                                     name=f"E8_{h}_{mp}_{ci}")
                        exp_pair(PQ[:, 0:2 * w], E8[:, 0:2 * w])
                        E8s[h] = E8
                    if hp == 0 and ci == 0:
                        v_proj(2 * mp)
                        v_proj(2 * mp + 1)
                    for h in heads:
                        mmDR(U[h][:, 0:w], V_v[:, mp, :, h, 0:D + 1],
                             rhs=E8s[h][:, 0:2 * w].rearrange(
                                 "p (s q) -> p s q", s=2),
                             start=(mp == 0), stop=(mp == 3))
                    # interleave remaining chunk-1 projections of q0/k0 and
                    # next-phase projections / out-proj partials
                    if hp == 0 and ci == 0 and mp == 1:
                        proj_t(wq_t, xq_t, tbt_sb[:, 0:1], 0, qT, 0.125,
                               chunks=(1,))
                    if hp == 0 and ci == 0 and mp == 3:
                        proj_t(wk_t, xk_t, tbt_sb[:, 4:5], 0, kT, 1.0,
                               chunks=(1,))
                    if hp < 3 and mp == 2:
                        if ci == 0:
                            proj_t(wq_t, xq_t, tbt_sb[:, hp + 1:hp + 2],
                                   hp + 1, nxt["q"], 0.125)
                        else:
                            proj_t(wk_t, xk_t, tbt_sb[:, 5 + hp:6 + hp],
                                   hp + 1, nxt["k"], 1.0)
                    if hp == 3 and ci == 0:
                        out_proj_partial(mp + 4)
                    if hp == 3 and ci == 1:
                        out_proj(mp)
                # per-chunk softmax normalization (denominator in U row D)
                for h in heads:
                    rc = yp.tile([1, 512], f32, tag="rc")
                    nc.vector.reciprocal(rc[:, 0:w], U[h][D:D + 1, 0:w])
                    B = yp.tile([64, 512], f32, tag="B")
                    nc.gpsimd.partition_broadcast(B[:, 0:w], rc[0:1, 0:w],
                                                  channels=64)
                    base = (h % 2) * 64
                    nc.vector.tensor_mul(oT[hp][base:base + 64, off:off + w],
                                         U[h][0:D, 0:w], B[:, 0:w])
                if hp == 3 and ci >= 1:
                    for m in range(off // 128, (off + w) // 128):
                        out_proj_final(m)

        nxt = {"q": qk_tiles["qA"], "k": qk_tiles["kA"]}
        proj_t(wq_t, xq_t, tbt_sb[:, 0:1], 0, nxt["q"], 0.125, chunks=(0,))
        proj_t(wk_t, xk_t, tbt_sb[:, 4:5], 0, nxt["k"], 1.0, chunks=(0,),
               split_epi=True)
        for hp in range(4):
            cur_q, cur_k = nxt["q"], nxt["k"]
            nxt = {"q": qk_tiles["qB" if hp % 2 == 0 else "qA"],
                   "k": qk_tiles["kB" if hp % 2 == 0 else "kA"]}
            attention(hp, cur_q, cur_k)

    nc.compile()
    return nc


def get_nc():
    if "nc" not in _CACHE:
        _CACHE["nc"] = _build()
    return _CACHE["nc"]


def _prep_inputs(query, key, value, Wq, Wk, Wv, Wo, bn_params):
    """Host-side: shard + transpose + fold BN scale into weights + fp8."""
    import ml_dtypes

    f8 = ml_dtypes.float8_e4m3

    query = np.ascontiguousarray(np.asarray(query, dtype=np.float32))
    key = np.ascontiguousarray(np.asarray(key, dtype=np.float32))
    value = np.ascontiguousarray(np.asarray(value, dtype=np.float32))
    bn = np.asarray(bn_params, dtype=np.float32)

    s = bn[:, 0] / np.sqrt(bn[:, 3] + EPS)      # [4, C]
    t = bn[:, 1] - bn[:, 2] * s                  # [4, C]

    def wprep(W, j, scale=1.0):
        W = np.asarray(W, dtype=np.float32)
        return np.ascontiguousarray((W * (s[j] * scale)[:, None]).T)

    wq8 = wprep(Wq, 0, 0.125).astype(f8)
    wk8 = wprep(Wk, 1).astype(f8)
    wv8 = wprep(Wv, 2).astype(f8)
    woT = wprep(Wo, 3)
    tbias = np.ascontiguousarray(t)
    # transposed q/k biases: rows (proj, d-tile) of 128; q scaled by 1/8
    tbt = np.ascontiguousarray(
        np.concatenate([(t[0] * 0.125).reshape(4, 128),
                        t[1].reshape(4, 128)]).T)

    # [T, B, N, C] -> [8, C, N] fp8
    def xT(x):
        return np.ascontiguousarray(
            x.reshape(N_CORES, N, C).transpose(0, 2, 1)).astype(f8)

    qT, kT, vT = xT(query), xT(key), xT(value)

    in_maps = []
    for i in range(N_CORES):
        in_maps.append({
            "xq": qT[i], "xk": kT[i], "xv": vT[i],
            "wq": wq8, "wk": wk8, "wv": wv8, "wo": woT,
            "tbias": tbias, "tbt": tbt,
        })
    return in_maps


def kernel(query, key, value, Wq, Wk, Wv, Wo, bn_params):
    from concourse.bass_utils import run_bass_kernel_spmd

    nc = get_nc()
    in_maps = _prep_inputs(query, key, value, Wq, Wk, Wv, Wo, bn_params)
    res = run_bass_kernel_spmd(nc, in_maps, core_ids=list(range(N_CORES)),
                               trace=False)
    T, B = 4, 2
    out = np.stack([res.results[i]["out"] for i in range(N_CORES)])
    return np.ascontiguousarray(out.reshape(T, B, N, C).astype(np.float32))
